# revision 11
# baseline (speedup 1.0000x reference)
"""Trainium2 Bass kernel for nn_CotLayer (CoT attention layer).

Computation (see reference):
  kemb = relu(grouped_conv3x3(x, Wk, groups=4))
  w1   = relu(We1 @ [x; kemb])            (1x1)
  wbar_k = We2_k @ w1 + be2_k             (per-pixel 3x3 kernel, 8-fold
                                           group replication folded into We2_k)
  xv   = Wv @ x                           (1x1)
  agg  = relu(sum_k shift_k(xv) * wbar_k)
  gap  = mean_{H,W}(agg + kemb)           (AllReduce across 4-core groups)
  attn0 = sigmoid(a0 - a1) from SE-MLP(gap); out = kemb + attn0*(agg - kemb)

Sharding: 8 cores = (batch b) x (H-quarter q); each core computes 64 output
rows; 1-px halo baked into its input slab host-side. x is pre-cast to bf16
on the host; all matmuls are bf16 with fp32 PSUM accumulation.

v2 design:
 - x slab resident in SBUF (4 chunked DMAs, big contiguous descriptors);
   per-tile rhs are views, no per-tile input DMA.
 - xv stored once in a column-padded [C,6,258] layout; all 9 taps are AP
   views (no shifted copies).
 - 5 of 9 wbar taps consumed straight from PSUM by a fused DVE
   scalar_tensor_tensor (+be2 then *xv); 4 taps ACT-drained (bias fused),
   products on DVE/GPSIMD.
 - blend uses out = kemb + attn0*diff with diff = relu(agg_acc) - kemb
   computed in phase 1 (accum_out gives the gap sums for free); bf16 out.
"""

import numpy as np
import ml_dtypes
from contextlib import ExitStack

import concourse.bass as bass
import concourse.tile as tile
from concourse import bacc, mybir
from concourse.bass_utils import run_bass_kernel_spmd

F32 = mybir.dt.float32
BF16 = mybir.dt.bfloat16
AL = mybir.AluOpType
AF = mybir.ActivationFunctionType
BF = ml_dtypes.bfloat16

B, C, H, W = 2, 128, 256, 256
KSZ, SP = 3, 8
NCORES = 8
RQ = H // 4          # 64 rows per core
TR = 4               # output rows per macro-tile
NT = RQ // TR        # 16 macro-tiles per core
NPX = TR * W         # 1024 px per macro-tile

# ---- tuning knobs ----
# taps whose product reads wbar directly from PSUM via fused DVE STT
PSUM_STT_TAPS = (1, 2, 3, 4, 5, 7)
# ACT-drained taps -> plain tensor_tensor product on this engine
TT_ENGINE = {0: "vector", 6: "gpsimd", 8: "vector"}
GPS_ADDS = (2, 3)        # which of the 8 adds run on GPSIMD
GPS_BLEND_PERIOD = 4     # every Nth phase-2 blend op goes via ACT+GPSIMD


def _prep_weights(inputs):
    Wk = np.asarray(inputs["Wk"], np.float32)
    We1 = np.asarray(inputs["We1"], np.float32)[:, :, 0, 0]
    We2 = np.asarray(inputs["We2"], np.float32)[:, :, 0, 0]
    be2 = np.asarray(inputs["be2"], np.float32)
    Wv = np.asarray(inputs["Wv"], np.float32)[:, :, 0, 0]
    Ws1 = np.asarray(inputs["Ws1"], np.float32)[:, :, 0, 0]
    bs1 = np.asarray(inputs["bs1"], np.float32)
    Ws2 = np.asarray(inputs["Ws2"], np.float32)[:, :, 0, 0]
    bs2 = np.asarray(inputs["bs2"], np.float32)

    wk = np.zeros((C, 9, C), np.float32)
    for t in range(9):
        a, b = divmod(t, 3)
        for g in range(4):
            blk = Wk[32 * g:32 * g + 32, :, a, b]
            wk[32 * g:32 * g + 32, t, 32 * g:32 * g + 32] = blk.T
    cidx = (np.arange(C) // SP) * 9
    we2 = np.zeros((64, 9, C), np.float32)
    be2k = np.zeros((C, 9), np.float32)
    for t in range(9):
        we2[:, t, :] = We2[cidx + t, :].T
        be2k[:, t] = be2[cidx + t]
    # taps packed pairwise into disjoint 64-row PE groups: even tap at
    # partitions 0-63, odd tap at 64-127 (reads the duplicated w1 half)
    we2p = np.zeros((C, 5, C), np.float32)
    for jj in range(5):
        we2p[0:64, jj, :] = we2[:, 2 * jj, :]
        if jj < 4:
            we2p[64:C, jj, :] = we2[:, 2 * jj + 1, :]
    ws2 = np.zeros((64, 2, C), np.float32)
    ws2[:, 0, :] = Ws2[0::2, :].T
    ws2[:, 1, :] = Ws2[1::2, :].T
    bs2r = np.zeros((C, 2), np.float32)
    bs2r[:, 0] = bs2[0::2]
    bs2r[:, 1] = bs2[1::2]
    w1x2 = np.concatenate([We1[:, :C].T, We1[:, :C].T], axis=1)   # [128,128]
    w1k2 = np.concatenate([We1[:, C:].T, We1[:, C:].T], axis=1)   # [128,128]
    return dict(
        wk=np.ascontiguousarray(wk.astype(BF)),
        w1x=np.ascontiguousarray(w1x2.astype(BF)),
        w1k=np.ascontiguousarray(w1k2.astype(BF)),
        we2=np.ascontiguousarray(we2p.astype(BF)),
        be2=np.ascontiguousarray(be2k),
        wv=np.ascontiguousarray(Wv.T.astype(BF)),
        ws1=np.ascontiguousarray((Ws1.T / float(H * W)).astype(np.float32)),
        bs1=bs1.reshape(64, 1),
        ws2=np.ascontiguousarray(ws2),
        bs2=bs2r,
    )


def _build_kernel(nc):
    xs = nc.dram_tensor("xs", [C, RQ + 2, W + 2], BF16, kind="ExternalInput")
    wk_d = nc.dram_tensor("wk", [C, 9, C], BF16, kind="ExternalInput")
    w1x_d = nc.dram_tensor("w1x", [C, C], BF16, kind="ExternalInput")
    w1k_d = nc.dram_tensor("w1k", [C, C], BF16, kind="ExternalInput")
    we2_d = nc.dram_tensor("we2", [C, 5, C], BF16, kind="ExternalInput")
    be2_d = nc.dram_tensor("be2", [C, 9], F32, kind="ExternalInput")
    wv_d = nc.dram_tensor("wv", [C, C], BF16, kind="ExternalInput")
    ws1_d = nc.dram_tensor("ws1", [C, 64], F32, kind="ExternalInput")
    bs1_d = nc.dram_tensor("bs1", [64, 1], F32, kind="ExternalInput")
    ws2_d = nc.dram_tensor("ws2", [64, 2, C], F32, kind="ExternalInput")
    bs2_d = nc.dram_tensor("bs2", [C, 2], F32, kind="ExternalInput")
    out_d = nc.dram_tensor("out", [C, RQ * W], BF16, kind="ExternalOutput")

    cc_in = nc.dram_tensor("cc_in", [C, 1], F32, kind="Internal")
    cc_out = nc.dram_tensor("cc_out", [C, 1], F32, kind="Internal")

    with tile.TileContext(nc) as tc, ExitStack() as ctx:
        singles = ctx.enter_context(tc.tile_pool(name="singles", bufs=1))
        w1pool = ctx.enter_context(tc.tile_pool(name="w1p", bufs=3))
        xvpool = ctx.enter_context(tc.tile_pool(name="xvp", bufs=2))
        wbpool = ctx.enter_context(tc.tile_pool(name="wbp", bufs=2))
        prodp = ctx.enter_context(tc.tile_pool(name="prodp", bufs=2))
        accp = ctx.enter_context(tc.tile_pool(name="accp", bufs=5))
        outp = ctx.enter_context(tc.tile_pool(name="outp", bufs=3))
        smallp = ctx.enter_context(tc.tile_pool(name="smallp", bufs=1))
        # PSUM: pKW 2 x [128,1024] = 4 banks; pWB 2 x [128,1024] = 4 banks.
        pKW = ctx.enter_context(tc.tile_pool(name="pkw", bufs=2, space="PSUM"))
        pWB = ctx.enter_context(tc.tile_pool(name="pwb", bufs=2, space="PSUM"))

        def sb(name, shape, dt, dram):
            t_ = singles.tile(shape, dt, tag=name)
            nc.sync.dma_start(t_, dram.ap())
            return t_

        wk_sb = sb("wk", [C, 9, C], BF16, wk_d)
        w1x_sb = sb("w1x", [C, C], BF16, w1x_d)
        w1k_sb = sb("w1k", [C, C], BF16, w1k_d)
        we2_sb = sb("we2", [C, 5, C], BF16, we2_d)
        be2_sb = sb("be2", [C, 9], F32, be2_d)
        wv_sb = sb("wv", [C, C], BF16, wv_d)
        ws1_sb = sb("ws1", [C, 64], F32, ws1_d)
        bs1_sb = sb("bs1", [64, 1], F32, bs1_d)
        ws2_sb = sb("ws2", [64, 2, C], F32, ws2_d)
        bs2_sb = sb("bs2", [C, 2], F32, bs2_d)

        # resident x slab: 4 overlapping row-chunks, each one big DMA
        xchunks = []
        for c4 in range(4):
            xt = singles.tile([C, 18, W + 2], BF16, tag=f"xs{c4}")
            nc.sync.dma_start(xt, xs.ap()[:, 16 * c4:16 * c4 + 18, :])
            xchunks.append(xt)

        kemb_slab = singles.tile([C, RQ * W], BF16)
        diff_slab = singles.tile([C, RQ * W], BF16)
        slots_k = singles.tile([C, NT], F32)
        slots_d = singles.tile([C, NT], F32)
        attn_sb = singles.tile([C, 1], F32)

        # pre-warm the sigmoid ACT table so the SE tail doesn't pay the
        # ~2.7us table load on the critical path
        warm = smallp.tile([C, 1], F32, tag="warm")
        nc.vector.memset(warm, 0.0)
        nc.scalar.activation(warm, warm, AF.Sigmoid)

        # ---------------- phase 1 ----------------
        for t in range(NT):
            xc = xchunks[t // 4][:, 4 * (t % 4):4 * (t % 4) + 6, :]

            # kemb: grouped 3x3 conv as block-diag matmuls; one LDWEIGHTS
            # per tap (both pixel-halves run under it)
            pk = pKW.tile([C, NPX], F32, tag="kw")
            for tap in range(9):
                a, b = divmod(tap, 3)
                for g2 in range(2):
                    nc.tensor.matmul(
                        pk[:, g2 * 512:(g2 + 1) * 512],
                        lhsT=wk_sb[:, tap, :],
                        rhs=xc[:, 2 * g2 + a:2 * g2 + a + 2, b:b + W],
                        start=(tap == 0), stop=(tap == 8),
                    )
            kv = kemb_slab[:, t * NPX:(t + 1) * NPX]
            nc.scalar.activation(kv, pk, AF.Relu,
                                 accum_out=slots_k[:, t:t + 1])

            # w1 = relu(We1 @ [x; kemb]), duplicated into both 64-row halves
            pw = pKW.tile([C, NPX], F32, tag="kw")
            nc.tensor.matmul(pw[:, 0:512], lhsT=w1x_sb,
                             rhs=xc[:, 1:3, 1:1 + W], start=True, stop=False)
            nc.tensor.matmul(pw[:, 512:1024], lhsT=w1x_sb,
                             rhs=xc[:, 3:5, 1:1 + W], start=True, stop=False)
            nc.tensor.matmul(pw[:, 0:512], lhsT=w1k_sb, rhs=kv[:, 0:512],
                             start=False, stop=True)
            nc.tensor.matmul(pw[:, 512:1024], lhsT=w1k_sb, rhs=kv[:, 512:1024],
                             start=False, stop=True)
            w1b = w1pool.tile([C, NPX], BF16, tag="w1")
            nc.scalar.activation(w1b, pw, AF.Relu)

            # last tile keeps GPSIMD free so the collective triggers promptly
            gps_ok = (t != NT - 1)

            # xv = Wv @ x over 6 rows -> contiguous xvc; shifted copies
            # xvl/xvr on GPSIMD so tap products stay in DVE 2x mode
            xvc = xvpool.tile([C, TR + 2, W], BF16, tag="xvc")
            pxv01 = pKW.tile([C, NPX], F32, tag="kw")
            nc.tensor.matmul(pxv01[:, 0:512], lhsT=wv_sb,
                             rhs=xc[:, 0:2, 1:1 + W], start=True, stop=True)
            nc.tensor.matmul(pxv01[:, 512:1024], lhsT=wv_sb,
                             rhs=xc[:, 2:4, 1:1 + W], start=True, stop=True)
            nc.scalar.activation(xvc[:, 0:4, :],
                                 pxv01.rearrange("p (r w) -> p r w", w=W),
                                 AF.Copy)
            pxv2 = pKW.tile([C, 512], F32, tag="kw")
            nc.tensor.matmul(pxv2, lhsT=wv_sb,
                             rhs=xc[:, 4:6, 1:1 + W], start=True, stop=True)
            nc.scalar.activation(xvc[:, 4:6, :],
                                 pxv2.rearrange("p (r w) -> p r w", w=W),
                                 AF.Copy)
            xvl = xvpool.tile([C, TR + 2, W], BF16, tag="xvl")
            xvr = xvpool.tile([C, TR + 2, W], BF16, tag="xvr")
            ceng = nc.gpsimd if gps_ok else nc.vector
            ceng.tensor_copy(xvl[:, :, 0:W - 1], xvc[:, :, 1:W])
            nc.gpsimd.memset(xvl[:, :, W - 1:W], 0.0)
            ceng.tensor_copy(xvr[:, :, 1:W], xvc[:, :, 0:W - 1])
            nc.gpsimd.memset(xvr[:, :, 0:1], 0.0)
            srcs = {0: xvr, 1: xvc, 2: xvl}

            # wbar pairs into full-tile PSUM tap tiles (h-major so the
            # 64-row-group pair MMs run concurrently); products fused from
            # PSUM (STT) or ACT-drained + TT
            prods = {}
            for jj in range(5):
                taps = [2 * jj] + ([2 * jj + 1] if jj < 4 else [])
                pbs = {}
                for tap in taps:
                    pbs[tap] = pWB.tile([C, NPX], F32, name=f"pb{tap}",
                                        tag="wb")
                for h in range(2):
                    for ti, tap in enumerate(taps):
                        lo = 64 * ti
                        nc.tensor.matmul(
                            pbs[tap][:, h * 512:h * 512 + 512],
                            lhsT=we2_sb[lo:lo + 64, jj, :],
                            rhs=w1b[lo:lo + 64, h * 512:h * 512 + 512],
                            start=True, stop=True)
                for tap in taps:
                    a, b = divmod(tap, 3)
                    xvv = srcs[b][:, a:a + TR, :]
                    pbv = pbs[tap].rearrange("p (r w) -> p r w", w=W)
                    p = prodp.tile([C, TR, W], BF16, name=f"p{tap}",
                                   tag=f"p{tap}")
                    if tap in PSUM_STT_TAPS:
                        nc.vector.scalar_tensor_tensor(
                            p, pbv, be2_sb[:, tap:tap + 1], xvv,
                            AL.add, AL.mult)
                    else:
                        wb = wbpool.tile([C, TR, W], BF16, name=f"wb{tap}",
                                         tag=f"wb{tap}")
                        nc.scalar.activation(wb, pbv, AF.Identity,
                                             bias=be2_sb[:, tap:tap + 1])
                        eng = getattr(nc, TT_ENGINE[tap])
                        if eng is nc.gpsimd and not gps_ok:
                            eng = nc.vector
                        eng.tensor_tensor(p, xvv, wb, AL.mult)
                    prods[tap] = p

            # add tree
            def add(i, x1, x2):
                s = accp.tile([C, TR, W], BF16, tag="acc")
                eng = nc.gpsimd if (i in GPS_ADDS and gps_ok) else nc.vector
                eng.tensor_tensor(s, x1, x2, AL.add)
                return s

            s1 = add(0, prods[0], prods[1])
            s2 = add(1, prods[2], prods[3])
            s3 = add(2, prods[4], prods[5])
            s4 = add(3, prods[6], prods[7])
            s5 = add(4, s1, s2)
            s6 = add(5, s3, prods[8])
            s7 = add(6, s5, s6)
            acc = add(7, s7, s4)

            # diff = relu(acc) - kemb; accum gives sum(diff) for the gap
            dv = diff_slab[:, t * NPX:(t + 1) * NPX]
            accf = acc.rearrange("p r w -> p (r w)")
            nc.vector.scalar_tensor_tensor(
                dv, accf, 0.0, kv, AL.max, AL.subtract,
                accum_out=slots_d[:, t:t + 1])

        # ---------------- SE attention (tiny) ----------------
        sum_k = smallp.tile([C, 1], F32, tag="sk")
        sum_d = smallp.tile([C, 1], F32, tag="sd")
        nc.vector.tensor_reduce(sum_k, slots_k, mybir.AxisListType.X, AL.add)
        nc.vector.tensor_reduce(sum_d, slots_d, mybir.AxisListType.X, AL.add)
        # gap_pre = 2*sum(kemb) + sum(diff)   (mean folded into ws1)
        gap = smallp.tile([C, 1], F32, tag="gap")
        nc.vector.scalar_tensor_tensor(gap, sum_k, 2.0, sum_d,
                                       AL.mult, AL.add)
        nc.gpsimd.dma_start(cc_in.ap(), gap)
        nc.gpsimd.collective_compute(
            "AllReduce", AL.add,
            replica_groups=[[0, 1, 2, 3], [4, 5, 6, 7]],
            ins=[cc_in.ap().opt()],
            outs=[cc_out.ap().opt()],
        )
        gap2 = smallp.tile([C, 1], F32, tag="gap2")
        nc.gpsimd.dma_start(gap2, cc_out.ap())

        ph = pKW.tile([64, 1], F32, tag="kw")
        nc.tensor.matmul(ph, lhsT=ws1_sb, rhs=gap2, start=True, stop=True)
        hso = smallp.tile([64, 1], F32, tag="h")
        nc.scalar.activation(hso, ph, AF.Relu, bias=bs1_sb[:, 0:1])
        pa = pKW.tile([C, 2], F32, tag="kw")
        nc.tensor.matmul(pa[:, 0:1], lhsT=ws2_sb[:, 0, :], rhs=hso,
                         start=True, stop=True)
        nc.tensor.matmul(pa[:, 1:2], lhsT=ws2_sb[:, 1, :], rhs=hso,
                         start=True, stop=True)
        a01 = smallp.tile([C, 2], F32, tag="a01")
        nc.scalar.activation(a01[:, 0:1], pa[:, 0:1], AF.Identity,
                             bias=bs2_sb[:, 0:1])
        nc.scalar.activation(a01[:, 1:2], pa[:, 1:2], AF.Identity,
                             bias=bs2_sb[:, 1:2])
        dse = smallp.tile([C, 1], F32, tag="dse")
        nc.vector.tensor_tensor(dse, a01[:, 0:1], a01[:, 1:2], AL.subtract)
        nc.scalar.activation(attn_sb[:, 0:1], dse, AF.Sigmoid)

        # ---------------- phase 2: blend + store ----------------
        # out = kemb + attn0 * diff; GPSIMD lacks AP-scalar ops, so its
        # share of tiles is split ACT (scale-copy) + GPSIMD (add)
        for t in range(NT):
            kv = kemb_slab[:, t * NPX:(t + 1) * NPX]
            dvv = diff_slab[:, t * NPX:(t + 1) * NPX]
            outb = outp.tile([C, NPX], BF16, tag="outb")
            if (t % GPS_BLEND_PERIOD) == (GPS_BLEND_PERIOD - 1):
                t1 = outp.tile([C, NPX], BF16, tag="t1")
                nc.scalar.activation(t1, dvv, AF.Copy, scale=attn_sb[:, 0:1])
                nc.gpsimd.tensor_tensor(outb, t1, kv, AL.add)
            else:
                nc.vector.scalar_tensor_tensor(outb, dvv, attn_sb[:, 0:1],
                                               kv, AL.mult, AL.add)
            nc.sync.dma_start(out_d.ap()[:, t * NPX:(t + 1) * NPX], outb)

    return nc


_CACHE = {}


def _get_nc():
    if "nc" not in _CACHE:
        nc = bacc.Bacc("TRN2", target_bir_lowering=False, debug=False,
                       num_devices=NCORES)
        _build_kernel(nc)
        nc.compile()
        _CACHE["nc"] = nc
    return _CACHE["nc"]


def make_in_maps(inputs):
    x = np.asarray(inputs["x"], np.float32)
    wts = _prep_weights(inputs)
    xp = np.pad(x, ((0, 0), (0, 0), (1, 1), (1, 1))).astype(BF)
    in_maps = []
    for core in range(NCORES):
        bb, q = divmod(core, 4)
        slab = np.ascontiguousarray(xp[bb, :, RQ * q:RQ * q + RQ + 2, :])
        m = {"xs": slab}
        m.update(wts)
        in_maps.append(m)
    return in_maps


def kernel(**inputs):
    in_maps = make_in_maps(inputs)
    nc = _get_nc()
    res = run_bass_kernel_spmd(nc, in_maps, core_ids=list(range(NCORES)))
    out = np.empty((B, C, H, W), np.float32)
    for core in range(NCORES):
        bb, q = divmod(core, 4)
        out[bb, :, RQ * q:RQ * q + RQ, :] = \
            res.results[core]["out"].astype(np.float32).reshape(C, RQ, W)
    return out


# revision 16
# speedup vs baseline: 1.5425x; 1.5425x over previous
"""Trainium2 Bass kernel for nn_CotLayer (CoT attention layer).

Computation (see reference):
  kemb = relu(grouped_conv3x3(x, Wk, groups=4))
  w1   = relu(We1 @ [x; kemb])            (1x1)
  wbar_k = We2_k @ w1 + be2_k             (per-pixel 3x3 kernel, 8-fold
                                           group replication folded into We2_k)
  xv   = Wv @ x                           (1x1)
  agg  = relu(sum_k shift_k(xv) * wbar_k)
  gap  = mean_{H,W}(agg + kemb)           (AllReduce across 4-core groups)
  attn0 = sigmoid(a0 - a1) from SE-MLP(gap); out = kemb + attn0*(agg - kemb)

Sharding: 8 cores = (batch b) x (H-quarter q); each core computes 64 output
rows; 1-px halo baked into its input slab host-side. x is pre-cast to bf16
on the host; all matmuls are bf16 with fp32 PSUM accumulation.

v2 design:
 - x slab resident in SBUF (4 chunked DMAs, big contiguous descriptors);
   per-tile rhs are views, no per-tile input DMA.
 - xv stored once in a column-padded [C,6,258] layout; all 9 taps are AP
   views (no shifted copies).
 - 5 of 9 wbar taps consumed straight from PSUM by a fused DVE
   scalar_tensor_tensor (+be2 then *xv); 4 taps ACT-drained (bias fused),
   products on DVE/GPSIMD.
 - blend uses out = kemb + attn0*diff with diff = relu(agg_acc) - kemb
   computed in phase 1 (accum_out gives the gap sums for free); bf16 out.
"""

import numpy as np
import ml_dtypes
from contextlib import ExitStack

import concourse.bass as bass
import concourse.tile as tile
from concourse import bacc, mybir
from concourse.bass_utils import run_bass_kernel_spmd

F32 = mybir.dt.float32
BF16 = mybir.dt.bfloat16
AL = mybir.AluOpType
AF = mybir.ActivationFunctionType
BF = ml_dtypes.bfloat16

B, C, H, W = 2, 128, 256, 256
KSZ, SP = 3, 8
NCORES = 8
RQ = H // 4          # 64 rows per core
TR = 4               # output rows per macro-tile
NT = RQ // TR        # 16 macro-tiles per core
NPX = TR * W         # 1024 px per macro-tile

# ---- tuning knobs ----
# taps whose product reads wbar directly from PSUM via fused DVE STT
# (b=1 taps 1,4 are ACT-drained + DVE TT from contiguous xvc instead)
PSUM_STT_TAPS = (0, 2, 3, 5, 6, 7, 8)
# products accumulated on PE via identity-matmul PSUM chain (rest on DVE tree)
PE_ACC_PRODS = (0, 1, 2, 3, 4, 5)
GPS_BLEND_PERIOD = 4     # every Nth phase-2 blend op goes via ACT+GPSIMD


def _prep_weights(inputs):
    Wk = np.asarray(inputs["Wk"], np.float32)
    We1 = np.asarray(inputs["We1"], np.float32)[:, :, 0, 0]
    We2 = np.asarray(inputs["We2"], np.float32)[:, :, 0, 0]
    be2 = np.asarray(inputs["be2"], np.float32)
    Wv = np.asarray(inputs["Wv"], np.float32)[:, :, 0, 0]
    Ws1 = np.asarray(inputs["Ws1"], np.float32)[:, :, 0, 0]
    bs1 = np.asarray(inputs["bs1"], np.float32)
    Ws2 = np.asarray(inputs["Ws2"], np.float32)[:, :, 0, 0]
    bs2 = np.asarray(inputs["bs2"], np.float32)

    wk = np.zeros((C, 9, C), np.float32)
    for t in range(9):
        a, b = divmod(t, 3)
        for g in range(4):
            blk = Wk[32 * g:32 * g + 32, :, a, b]
            wk[32 * g:32 * g + 32, t, 32 * g:32 * g + 32] = blk.T
    cidx = (np.arange(C) // SP) * 9
    we2 = np.zeros((64, 9, C), np.float32)
    be2k = np.zeros((C, 9), np.float32)
    for t in range(9):
        we2[:, t, :] = We2[cidx + t, :].T
        be2k[:, t] = be2[cidx + t]
    # taps packed pairwise into disjoint 64-row PE groups: even tap at
    # partitions 0-63, odd tap at 64-127 (reads the duplicated w1 half)
    we2p = np.zeros((C, 5, C), np.float32)
    for jj in range(5):
        we2p[0:64, jj, :] = we2[:, 2 * jj, :]
        if jj < 4:
            we2p[64:C, jj, :] = we2[:, 2 * jj + 1, :]
    ws2 = np.zeros((64, 2, C), np.float32)
    ws2[:, 0, :] = Ws2[0::2, :].T
    ws2[:, 1, :] = Ws2[1::2, :].T
    bs2r = np.zeros((C, 2), np.float32)
    bs2r[:, 0] = bs2[0::2]
    bs2r[:, 1] = bs2[1::2]
    w1x2 = np.concatenate([We1[:, :C].T, We1[:, :C].T], axis=1)   # [128,128]
    w1k2 = np.concatenate([We1[:, C:].T, We1[:, C:].T], axis=1)   # [128,128]
    return dict(
        wk=np.ascontiguousarray(wk.astype(BF)),
        w1x=np.ascontiguousarray(w1x2.astype(BF)),
        w1k=np.ascontiguousarray(w1k2.astype(BF)),
        we2=np.ascontiguousarray(we2p.astype(BF)),
        be2=np.ascontiguousarray(be2k),
        wv=np.ascontiguousarray(Wv.T.astype(BF)),
        ws1=np.ascontiguousarray((Ws1.T / float(H * W)).astype(np.float32)),
        bs1=bs1.reshape(64, 1),
        ws2=np.ascontiguousarray(ws2),
        bs2=bs2r,
        ident=np.ascontiguousarray(np.eye(C, dtype=np.float32).astype(BF)),
    )


def _build_kernel(nc):
    xs = nc.dram_tensor("xs", [C, RQ + 2, W + 2], BF16, kind="ExternalInput")
    wk_d = nc.dram_tensor("wk", [C, 9, C], BF16, kind="ExternalInput")
    w1x_d = nc.dram_tensor("w1x", [C, C], BF16, kind="ExternalInput")
    w1k_d = nc.dram_tensor("w1k", [C, C], BF16, kind="ExternalInput")
    we2_d = nc.dram_tensor("we2", [C, 5, C], BF16, kind="ExternalInput")
    be2_d = nc.dram_tensor("be2", [C, 9], F32, kind="ExternalInput")
    wv_d = nc.dram_tensor("wv", [C, C], BF16, kind="ExternalInput")
    ws1_d = nc.dram_tensor("ws1", [C, 64], F32, kind="ExternalInput")
    bs1_d = nc.dram_tensor("bs1", [64, 1], F32, kind="ExternalInput")
    ws2_d = nc.dram_tensor("ws2", [64, 2, C], F32, kind="ExternalInput")
    bs2_d = nc.dram_tensor("bs2", [C, 2], F32, kind="ExternalInput")
    id_d = nc.dram_tensor("ident", [C, C], BF16, kind="ExternalInput")
    out_d = nc.dram_tensor("out", [C, RQ * W], BF16, kind="ExternalOutput")

    cc_in = nc.dram_tensor("cc_in", [C, 1], F32, kind="Internal")
    cc_out = nc.dram_tensor("cc_out", [C, 1], F32, kind="Internal")

    with tile.TileContext(nc) as tc, ExitStack() as ctx:
        singles = ctx.enter_context(tc.tile_pool(name="singles", bufs=1))
        w1pool = ctx.enter_context(tc.tile_pool(name="w1p", bufs=3))
        xvpool = ctx.enter_context(tc.tile_pool(name="xvp", bufs=2))
        wbpool = ctx.enter_context(tc.tile_pool(name="wbp", bufs=2))
        prodp = ctx.enter_context(tc.tile_pool(name="prodp", bufs=2))
        accp = ctx.enter_context(tc.tile_pool(name="accp", bufs=5))
        outp = ctx.enter_context(tc.tile_pool(name="outp", bufs=3))
        smallp = ctx.enter_context(tc.tile_pool(name="smallp", bufs=1))
        # PSUM: pKW 2 x [128,1024] = 4 banks; pWB 2 x [128,1024] = 4 banks.
        pKW = ctx.enter_context(tc.tile_pool(name="pkw", bufs=2, space="PSUM"))
        pWB = ctx.enter_context(tc.tile_pool(name="pwb", bufs=2, space="PSUM"))

        def sb(name, shape, dt, dram):
            t_ = singles.tile(shape, dt, tag=name)
            nc.sync.dma_start(t_, dram.ap())
            return t_

        wk_sb = sb("wk", [C, 9, C], BF16, wk_d)
        w1x_sb = sb("w1x", [C, C], BF16, w1x_d)
        w1k_sb = sb("w1k", [C, C], BF16, w1k_d)
        we2_sb = sb("we2", [C, 5, C], BF16, we2_d)
        be2_sb = sb("be2", [C, 9], F32, be2_d)
        wv_sb = sb("wv", [C, C], BF16, wv_d)
        ws1_sb = sb("ws1", [C, 64], F32, ws1_d)
        bs1_sb = sb("bs1", [64, 1], F32, bs1_d)
        ws2_sb = sb("ws2", [64, 2, C], F32, ws2_d)
        bs2_sb = sb("bs2", [C, 2], F32, bs2_d)
        id_sb = sb("ident", [C, C], BF16, id_d)

        # resident x slab: 4 overlapping row-chunks, each one big DMA
        xchunks = []
        for c4 in range(4):
            xt = singles.tile([C, 18, W + 2], BF16, tag=f"xs{c4}")
            nc.sync.dma_start(xt, xs.ap()[:, 16 * c4:16 * c4 + 18, :])
            xchunks.append(xt)

        kemb_slab = singles.tile([C, RQ * W], BF16)
        diff_slab = singles.tile([C, RQ * W], BF16)
        slots_k = singles.tile([C, NT], F32)
        slots_d = singles.tile([C, NT], F32)
        attn_sb = singles.tile([C, 1], F32)

        # pre-warm the sigmoid ACT table so the SE tail doesn't pay the
        # ~2.7us table load on the critical path
        warm = smallp.tile([C, 1], F32, tag="warm")
        nc.vector.memset(warm, 0.0)
        nc.scalar.activation(warm, warm, AF.Sigmoid)

        # ---------------- phase 1 ----------------
        for t in range(NT):
            xc = xchunks[t // 4][:, 4 * (t % 4):4 * (t % 4) + 6, :]

            # kemb: grouped 3x3 conv as block-diag matmuls; one LDWEIGHTS
            # per tap (both pixel-halves run under it)
            pk = pKW.tile([C, NPX], F32, tag="kw")
            for tap in range(9):
                a, b = divmod(tap, 3)
                for g2 in range(2):
                    nc.tensor.matmul(
                        pk[:, g2 * 512:(g2 + 1) * 512],
                        lhsT=wk_sb[:, tap, :],
                        rhs=xc[:, 2 * g2 + a:2 * g2 + a + 2, b:b + W],
                        start=(tap == 0), stop=(tap == 8),
                    )
            kv = kemb_slab[:, t * NPX:(t + 1) * NPX]
            nc.scalar.activation(kv, pk, AF.Relu,
                                 accum_out=slots_k[:, t:t + 1])

            # w1 = relu(We1 @ [x; kemb]), duplicated into both 64-row halves
            pw = pKW.tile([C, NPX], F32, tag="kw")
            nc.tensor.matmul(pw[:, 0:512], lhsT=w1x_sb,
                             rhs=xc[:, 1:3, 1:1 + W], start=True, stop=False)
            nc.tensor.matmul(pw[:, 512:1024], lhsT=w1x_sb,
                             rhs=xc[:, 3:5, 1:1 + W], start=True, stop=False)
            nc.tensor.matmul(pw[:, 0:512], lhsT=w1k_sb, rhs=kv[:, 0:512],
                             start=False, stop=True)
            nc.tensor.matmul(pw[:, 512:1024], lhsT=w1k_sb, rhs=kv[:, 512:1024],
                             start=False, stop=True)
            w1b = w1pool.tile([C, NPX], BF16, tag="w1")
            nc.scalar.activation(w1b, pw, AF.Relu)

            # xv = Wv @ x over 6 rows -> contiguous xvc (ACT from PSUM);
            # shifted xvl/xvr copies on ACT (products for b=1 taps read
            # contiguous xvc so their TTs stay in DVE 2x mode)
            xvc = xvpool.tile([C, TR + 2, W], BF16, tag="xvc")
            pxv01 = pKW.tile([C, NPX], F32, tag="kw")
            nc.tensor.matmul(pxv01[:, 0:512], lhsT=wv_sb,
                             rhs=xc[:, 0:2, 1:1 + W], start=True, stop=True)
            nc.tensor.matmul(pxv01[:, 512:1024], lhsT=wv_sb,
                             rhs=xc[:, 2:4, 1:1 + W], start=True, stop=True)
            nc.scalar.activation(xvc[:, 0:4, :],
                                 pxv01.rearrange("p (r w) -> p r w", w=W),
                                 AF.Copy)
            pxv2 = pKW.tile([C, 512], F32, tag="kw")
            nc.tensor.matmul(pxv2, lhsT=wv_sb,
                             rhs=xc[:, 4:6, 1:1 + W], start=True, stop=True)
            nc.scalar.activation(xvc[:, 4:6, :],
                                 pxv2.rearrange("p (r w) -> p r w", w=W),
                                 AF.Copy)
            xvl = xvpool.tile([C, TR + 2, W], BF16, tag="xvl")
            xvr = xvpool.tile([C, TR + 2, W], BF16, tag="xvr")
            nc.scalar.activation(xvl[:, :, 0:W - 1], xvc[:, :, 1:W], AF.Copy)
            nc.gpsimd.memset(xvl[:, :, W - 1:W], 0.0)
            nc.scalar.activation(xvr[:, :, 1:W], xvc[:, :, 0:W - 1], AF.Copy)
            nc.gpsimd.memset(xvr[:, :, 0:1], 0.0)
            srcs = {0: xvr, 1: xvc, 2: xvl}

            # wbar pairs into full-tile PSUM tap tiles (h-major so the
            # 64-row-group pair MMs run concurrently)
            pbs = {}
            for jj in range(5):
                taps = [2 * jj] + ([2 * jj + 1] if jj < 4 else [])
                for tap in taps:
                    pbs[tap] = pWB.tile([C, NPX], F32, name=f"pb{tap}",
                                        tag="wb")
                for h in range(2):
                    for ti, tap in enumerate(taps):
                        lo = 64 * ti
                        nc.tensor.matmul(
                            pbs[tap][:, h * 512:h * 512 + 512],
                            lhsT=we2_sb[lo:lo + 64, jj, :],
                            rhs=w1b[lo:lo + 64, h * 512:h * 512 + 512],
                            start=True, stop=True)
            # products: PSUM-direct fused STT on DVE, or ACT-drain + DVE TT
            prods = {}
            for tap in range(9):
                a, b = divmod(tap, 3)
                xvv = srcs[b][:, a:a + TR, :]
                pbv = pbs[tap].rearrange("p (r w) -> p r w", w=W)
                p = prodp.tile([C, TR, W], BF16, name=f"p{tap}",
                               tag=f"p{tap}")
                if tap in PSUM_STT_TAPS:
                    nc.vector.scalar_tensor_tensor(
                        p, pbv, be2_sb[:, tap:tap + 1], xvv,
                        AL.add, AL.mult)
                else:
                    wb = wbpool.tile([C, TR, W], BF16, name=f"wb{tap}",
                                     tag=f"wb{tap}")
                    nc.scalar.activation(wb, pbv, AF.Identity,
                                         bias=be2_sb[:, tap:tap + 1])
                    nc.vector.tensor_tensor(p, xvv, wb, AL.mult)
                prods[tap] = p

            # aggregation: PE identity-matmul accumulates most products in
            # PSUM; small DVE tree handles the rest, fed in as one operand
            rest = [prods[i] for i in range(9) if i not in PE_ACC_PRODS]
            sums = []
            while len(rest) > 1:
                s = accp.tile([C, TR, W], BF16, tag="acc")
                nc.vector.tensor_tensor(s, rest[0], rest[1], AL.add)
                rest = rest[2:] + [s]
            chain = [prods[i] for i in PE_ACC_PRODS] + rest
            pacc = pWB.tile([C, NPX], F32, name="pacc", tag="wb")
            for h in range(2):
                cs = slice(h * 512, h * 512 + 512)
                for ci, src in enumerate(chain):
                    sf = src.rearrange("p r w -> p (r w)")
                    nc.tensor.matmul(pacc[:, cs], lhsT=id_sb, rhs=sf[:, cs],
                                     start=(ci == 0),
                                     stop=(ci == len(chain) - 1))

            # diff = relu(acc) - kemb; accum gives sum(diff) for the gap
            dv = diff_slab[:, t * NPX:(t + 1) * NPX]
            nc.vector.scalar_tensor_tensor(
                dv, pacc, 0.0, kv, AL.max, AL.subtract,
                accum_out=slots_d[:, t:t + 1])

        # ---------------- SE attention (tiny) ----------------
        sum_k = smallp.tile([C, 1], F32, tag="sk")
        sum_d = smallp.tile([C, 1], F32, tag="sd")
        nc.vector.tensor_reduce(sum_k, slots_k, mybir.AxisListType.X, AL.add)
        nc.vector.tensor_reduce(sum_d, slots_d, mybir.AxisListType.X, AL.add)
        # gap_pre = 2*sum(kemb) + sum(diff)   (mean folded into ws1)
        gap = smallp.tile([C, 1], F32, tag="gap")
        nc.vector.scalar_tensor_tensor(gap, sum_k, 2.0, sum_d,
                                       AL.mult, AL.add)
        nc.gpsimd.dma_start(cc_in.ap(), gap)
        nc.gpsimd.collective_compute(
            "AllReduce", AL.add,
            replica_groups=[[0, 1, 2, 3], [4, 5, 6, 7]],
            ins=[cc_in.ap().opt()],
            outs=[cc_out.ap().opt()],
        )
        gap2 = smallp.tile([C, 1], F32, tag="gap2")
        nc.gpsimd.dma_start(gap2, cc_out.ap())

        ph = pKW.tile([64, 1], F32, tag="kw")
        nc.tensor.matmul(ph, lhsT=ws1_sb, rhs=gap2, start=True, stop=True)
        hso = smallp.tile([64, 1], F32, tag="h")
        nc.scalar.activation(hso, ph, AF.Relu, bias=bs1_sb[:, 0:1])
        pa = pKW.tile([C, 2], F32, tag="kw")
        nc.tensor.matmul(pa[:, 0:1], lhsT=ws2_sb[:, 0, :], rhs=hso,
                         start=True, stop=True)
        nc.tensor.matmul(pa[:, 1:2], lhsT=ws2_sb[:, 1, :], rhs=hso,
                         start=True, stop=True)
        a01 = smallp.tile([C, 2], F32, tag="a01")
        nc.scalar.activation(a01[:, 0:1], pa[:, 0:1], AF.Identity,
                             bias=bs2_sb[:, 0:1])
        nc.scalar.activation(a01[:, 1:2], pa[:, 1:2], AF.Identity,
                             bias=bs2_sb[:, 1:2])
        dse = smallp.tile([C, 1], F32, tag="dse")
        nc.vector.tensor_tensor(dse, a01[:, 0:1], a01[:, 1:2], AL.subtract)
        nc.scalar.activation(attn_sb[:, 0:1], dse, AF.Sigmoid)

        # ---------------- phase 2: blend + store ----------------
        # out = kemb + attn0 * diff; GPSIMD lacks AP-scalar ops, so its
        # share of tiles is split ACT (scale-copy) + GPSIMD (add)
        for t in range(NT):
            kv = kemb_slab[:, t * NPX:(t + 1) * NPX]
            dvv = diff_slab[:, t * NPX:(t + 1) * NPX]
            outb = outp.tile([C, NPX], BF16, tag="outb")
            if (t % GPS_BLEND_PERIOD) == (GPS_BLEND_PERIOD - 1):
                t1 = outp.tile([C, NPX], BF16, tag="t1")
                nc.scalar.activation(t1, dvv, AF.Copy, scale=attn_sb[:, 0:1])
                nc.gpsimd.tensor_tensor(outb, t1, kv, AL.add)
            else:
                nc.vector.scalar_tensor_tensor(outb, dvv, attn_sb[:, 0:1],
                                               kv, AL.mult, AL.add)
            nc.sync.dma_start(out_d.ap()[:, t * NPX:(t + 1) * NPX], outb)

    return nc


_CACHE = {}


def _get_nc():
    if "nc" not in _CACHE:
        nc = bacc.Bacc("TRN2", target_bir_lowering=False, debug=False,
                       num_devices=NCORES)
        _build_kernel(nc)
        nc.compile()
        _CACHE["nc"] = nc
    return _CACHE["nc"]


def make_in_maps(inputs):
    x = np.asarray(inputs["x"], np.float32)
    wts = _prep_weights(inputs)
    xp = np.pad(x, ((0, 0), (0, 0), (1, 1), (1, 1))).astype(BF)
    in_maps = []
    for core in range(NCORES):
        bb, q = divmod(core, 4)
        slab = np.ascontiguousarray(xp[bb, :, RQ * q:RQ * q + RQ + 2, :])
        m = {"xs": slab}
        m.update(wts)
        in_maps.append(m)
    return in_maps


def kernel(**inputs):
    in_maps = make_in_maps(inputs)
    nc = _get_nc()
    res = run_bass_kernel_spmd(nc, in_maps, core_ids=list(range(NCORES)))
    out = np.empty((B, C, H, W), np.float32)
    for core in range(NCORES):
        bb, q = divmod(core, 4)
        out[bb, :, RQ * q:RQ * q + RQ, :] = \
            res.results[core]["out"].astype(np.float32).reshape(C, RQ, W)
    return out
